# revision 1
# baseline (speedup 1.0000x reference)
"""Causal attention kernel for Trainium2, 8 NeuronCores, sequence-parallel.

Reference computation (T=4096, D=1024, fp32):
    q = x @ Wqk; logits = q @ x.T (causal masked); attn = softmax(logits)
    out = (attn @ x) @ Wov

Sharding: query rows split 512/core across 8 cores; Wqk/Wov replicated;
each core sees all keys (full x) and computes its row block end-to-end.

Per-core key blocks are permuted host-side so the causal structure is
core-independent: slot 0 = the diagonal 512-block (local triangular mask,
generated on device), slots 1..7 = the remaining blocks, with a per-core
additive bias beta in {0, -1e30} marking fully-visible / fully-masked
blocks. This keeps one SPMD program valid for every core.

Matmul precision: float32r (fp32 with 11-bit mantissa, exact fp32
accumulation) for q/scores/AV/Wov matmuls; softmax row max subtracted in
fp32; attn stored bf16 for the DMA-xbar transposes and AV matmul.
"""

import sys

sys.path.insert(0, "/opt/trn_rl_repo")

import numpy as np
import ml_dtypes

import concourse.tile as tile
from concourse import bacc, mybir
from concourse.bass_utils import run_bass_kernel_spmd

T = 4096
D = 1024
NCORES = 8
RQ = T // NCORES  # 512 query rows per core
NKB = T // 512  # 8 key slots of 512
KC = D // 128  # 8 contraction chunks
NMT = RQ // 128  # 4 query-row tiles per core
NEG = -1.0e30

f32 = mybir.dt.float32
f32r = mybir.dt.float32r
bf16 = mybir.dt.bfloat16


def _round_f32r(a: np.ndarray) -> np.ndarray:
    """Round fp32 to f32r encoding: RNE to 11 explicit mantissa bits."""
    u = np.ascontiguousarray(a, np.float32).view(np.uint32).astype(np.uint64)
    u = (u + 0x7FF + ((u >> 12) & 1)) & ~np.uint64(0xFFF)
    return u.astype(np.uint32).view(np.float32)


def _build_nc():
    nc = bacc.Bacc(
        "TRN2", target_bir_lowering=False, debug=False, num_devices=NCORES
    )

    xqt_d = nc.dram_tensor("xqt", [D, RQ], f32r, kind="ExternalInput").ap()
    xtp_d = nc.dram_tensor("xtp", [D, T], f32r, kind="ExternalInput").ap()
    xp_d = nc.dram_tensor("xp", [T, D], bf16, kind="ExternalInput").ap()
    wqk_d = nc.dram_tensor("wqk", [D, D], f32r, kind="ExternalInput").ap()
    wov_d = nc.dram_tensor("wov", [D, D], f32r, kind="ExternalInput").ap()
    beta_d = nc.dram_tensor("beta", [128, NKB], f32, kind="ExternalInput").ap()
    out_d = nc.dram_tensor("out", [RQ, D], f32, kind="ExternalOutput").ap()

    with tile.TileContext(nc) as tc:
        # stack allocator: allocate in order of decreasing lifetime
        consts = tc.alloc_tile_pool(name="consts", bufs=1)
        o1_pool = tc.alloc_tile_pool(name="o1pool", bufs=1)
        pt_pool = tc.alloc_tile_pool(name="ptpool", bufs=1)
        p_pool = tc.alloc_tile_pool(name="ppool", bufs=6)
        s_pool = tc.alloc_tile_pool(name="spool", bufs=NMT)
        qt_pool = tc.alloc_tile_pool(name="qt", bufs=1)
        xstream = tc.alloc_tile_pool(name="xstream", bufs=4)
        wqk_pool = tc.alloc_tile_pool(name="wqkp", bufs=1)

        # constants: stats scratch, tri mask, beta
        smalls = consts.tile([128, 68], f32, name="smalls")
        beta_sb = smalls[:, 0:NKB]
        nc.sync.dma_start(beta_sb, beta_d)
        tri = consts.tile([128, NMT * 512], bf16, name="tri")
        for mt in range(NMT):
            tm = tri[:, mt * 512 : (mt + 1) * 512]
            nc.gpsimd.memset(tm, 0.0)
            # keep 0 where (mt*128 + p - y) >= 0 i.e. key y <= local row; else -1e30
            nc.gpsimd.affine_select(
                out=tm,
                in_=tm,
                compare_op=mybir.AluOpType.is_ge,
                fill=NEG,
                base=mt * 128,
                pattern=[[-1, 512]],
                channel_multiplier=1,
            )
        negmax = smalls[:, 8:12]
        lsum = smalls[:, 12:16]
        recip = smalls[:, 16:20]
        mpart = smalls[:, 20:52]
        lq = smalls[:, 52:68]

        # ---- Phase A: qT = (xq @ Wqk)^T  -> [D, RQ] in f32r --------------
        xqt_sb = wqk_pool.tile([128, KC * RQ], f32r, name="xqt_sb")
        nc.sync.dma_start(
            xqt_sb.rearrange("p (kc n) -> p kc n", kc=KC),
            xqt_d.rearrange("(kc p) n -> p kc n", p=128),
        )
        qt_sb = qt_pool.tile([128, KC * RQ], f32r, name="qt_sb")

        with (
            tc.tile_pool(name="wqkstream", bufs=3) as wqkstream,
            tc.tile_pool(name="psA", bufs=2, space="PSUM") as psA,
        ):
            for mtd in range(KC):
                wqk_blk = wqkstream.tile([128, KC * 128], f32r, name="wqk_blk", tag="wq")
                nc.sync.dma_start(
                    wqk_blk.rearrange("p (kc n) -> p kc n", kc=KC),
                    wqk_d[:, mtd * 128 : (mtd + 1) * 128].rearrange(
                        "(kc p) n -> p kc n", p=128
                    ),
                )
                ps = psA.tile([128, RQ], f32, name="ps_qt")
                for kc in range(KC):
                    nc.tensor.matmul(
                        ps[:],
                        wqk_blk[:, kc * 128 : (kc + 1) * 128],
                        xqt_sb[:, kc * RQ : (kc + 1) * RQ],
                        start=(kc == 0),
                        stop=(kc == KC - 1),
                    )
                nc.vector.tensor_copy(qt_sb[:, mtd * RQ : (mtd + 1) * RQ], ps[:])
        wqk_pool.release()

        # ---- Phase B: scores S[mt] = qT^T @ xtp + mask -------------------
        s_tiles = [s_pool.tile([128, T], f32, name=f"s_mt{mt}", tag="s") for mt in range(NMT)]
        with tc.tile_pool(name="psB", bufs=2, space="PSUM") as psB:
            for kb in range(NKB):
                halves = []
                for hh in range(2):
                    xt_h = xstream.tile(
                        [128, (KC // 2) * 512], f32r, name="xt_h", tag="xt"
                    )
                    nc.sync.dma_start(
                        xt_h.rearrange("p (kc n) -> p kc n", kc=KC // 2),
                        xtp_d[
                            hh * (D // 2) : (hh + 1) * (D // 2),
                            kb * 512 : (kb + 1) * 512,
                        ].rearrange("(kc p) n -> p kc n", p=128),
                    )
                    halves.append(xt_h)
                for mt in range(NMT):
                    ps = psB.tile([128, 512], f32, name="ps_s")
                    for kc in range(KC):
                        nc.tensor.matmul(
                            ps[:],
                            qt_sb[:, kc * RQ + mt * 128 : kc * RQ + (mt + 1) * 128],
                            halves[kc // 4][:, (kc % 4) * 512 : (kc % 4 + 1) * 512],
                            start=(kc == 0),
                            stop=(kc == KC - 1),
                        )
                    dst = s_tiles[mt][:, kb * 512 : (kb + 1) * 512]
                    if kb == 0:
                        nc.vector.tensor_add(
                            dst, ps[:], tri[:, mt * 512 : (mt + 1) * 512]
                        )
                    else:
                        nc.vector.tensor_scalar_add(
                            dst, ps[:], beta_sb[:, kb : kb + 1]
                        )
                    nc.vector.tensor_reduce(
                        mpart[:, mt * NKB + kb : mt * NKB + kb + 1],
                        dst,
                        axis=mybir.AxisListType.X,
                        op=mybir.AluOpType.max,
                    )
                    if kb == NKB - 1:
                        # finalize this row tile's (negated) max immediately
                        # so exp can start while B's remaining tiles compute
                        nc.vector.tensor_reduce(
                            negmax[:, mt : mt + 1],
                            mpart[:, mt * NKB : (mt + 1) * NKB],
                            axis=mybir.AxisListType.X,
                            op=mybir.AluOpType.max,
                            negate=True,
                        )
        xstream.release()
        qt_pool.release()

        # ---- Phase C/D: exp in quarter chunks, pipelined with DMA xbar ---
        # transposes (out[p, kc, m] = in[m, kc*128 + p]) on the ACT HWDGE
        # ring, which must carry ONLY transposes: mixing plain copies onto
        # it corrupts transfers on this stack (hw xbar-mode hazard).
        QW = T // 4  # 1024 cols per exp/transpose chunk
        pt_tiles = [
            pt_pool.tile([128, 8 * RQ], bf16, name=f"pt_q{qq}", tag=f"ptq{qq}")
            for qq in range(4)
        ]
        pt_vs = [
            ptq.rearrange("p (kc four m) -> p kc four m", kc=8, four=NMT)
            for ptq in pt_tiles
        ]
        for qq in range(4):
            for mt in range(NMT):
                p_q = p_pool.tile([128, QW], bf16, name="p_q", tag="pq")
                nc.scalar.activation(
                    p_q[:],
                    s_tiles[mt][:, qq * QW : (qq + 1) * QW],
                    mybir.ActivationFunctionType.Exp,
                    bias=negmax[:, mt : mt + 1],
                    scale=1.0,
                    accum_out=lq[:, mt * 4 + qq : mt * 4 + qq + 1],
                )
                nc.scalar.dma_start_transpose(
                    pt_vs[qq][:, :, mt, :], p_q[:]
                )
        for mt in range(NMT):
            nc.vector.tensor_reduce(
                lsum[:, mt : mt + 1],
                lq[:, mt * 4 : (mt + 1) * 4],
                axis=mybir.AxisListType.X,
                op=mybir.AluOpType.add,
            )
            nc.vector.reciprocal(recip[:, mt : mt + 1], lsum[:, mt : mt + 1])
        s_pool.release()
        p_pool.release()
        wovstream = tc.alloc_tile_pool(name="wovstream", bufs=2)

        # ---- Phase E: o1T = xp^T @ attn^T  -> [D, RQ] f32r ---------------
        o1t_sb = o1_pool.tile([128, KC * RQ], f32r, name="o1t_sb")
        with (
            tc.tile_pool(name="xpstream", bufs=3) as xpstream,
            tc.tile_pool(name="psE", bufs=2, space="PSUM") as psE,
        ):
            for mtd in range(KC):
                xpb = xpstream.tile([128, (T // 128) * 128], bf16, name="xp_blk", tag="xp")
                nc.sync.dma_start(
                    xpb.rearrange("p (kc n) -> p kc n", kc=T // 128),
                    xp_d[:, mtd * 128 : (mtd + 1) * 128].rearrange(
                        "(kc p) n -> p kc n", p=128
                    ),
                )
                ps = psE.tile([128, RQ], f32, name="ps_av")
                for kc in range(T // 128):
                    nc.tensor.matmul(
                        ps[:],
                        xpb[:, kc * 128 : (kc + 1) * 128],
                        pt_tiles[kc // 8][:, (kc % 8) * RQ : (kc % 8 + 1) * RQ],
                        start=(kc == 0),
                        stop=(kc == T // 128 - 1),
                    )
                nc.vector.tensor_copy(o1t_sb[:, mtd * RQ : (mtd + 1) * RQ], ps[:])

        # ---- Phase F: out = (o1 @ Wov) * recip ---------------------------
        with (
            tc.tile_pool(name="psF", bufs=2, space="PSUM") as psF,
            tc.tile_pool(name="outp", bufs=3) as outp,
        ):
            for nb in range(2):
                wov_blk = wovstream.tile([128, KC * 512], f32r, name="wov_blk", tag="wv")
                nc.sync.dma_start(
                    wov_blk.rearrange("p (kc n) -> p kc n", kc=KC),
                    wov_d[:, nb * 512 : (nb + 1) * 512].rearrange(
                        "(kc p) n -> p kc n", p=128
                    ),
                )
                for mt in range(NMT):
                    ps = psF.tile([128, 512], f32, name="ps_o")
                    for kc in range(KC):
                        nc.tensor.matmul(
                            ps[:],
                            o1t_sb[:, kc * RQ + mt * 128 : kc * RQ + (mt + 1) * 128],
                            wov_blk[:, kc * 512 : (kc + 1) * 512],
                            start=(kc == 0),
                            stop=(kc == KC - 1),
                        )
                    ob = outp.tile([128, 512], f32, name="ob")
                    nc.vector.tensor_scalar_mul(
                        ob[:], ps[:], recip[:, mt : mt + 1]
                    )
                    nc.sync.dma_start(
                        out_d[mt * 128 : (mt + 1) * 128, nb * 512 : (nb + 1) * 512],
                        ob[:],
                    )

        wovstream.release()
        pt_pool.release()
        o1_pool.release()
        consts.release()

    nc.compile()
    return nc


_NC_CACHE = {}


def _get_nc():
    if "nc" not in _NC_CACHE:
        _NC_CACHE["nc"] = _build_nc()
    return _NC_CACHE["nc"]


def _prep_in_maps(x, Wqk, Wov):
    x = np.ascontiguousarray(np.asarray(x), dtype=np.float32)
    Wqk = np.ascontiguousarray(np.asarray(Wqk), dtype=np.float32)
    Wov = np.ascontiguousarray(np.asarray(Wov), dtype=np.float32)
    xT = np.ascontiguousarray(x.T)
    wqk_r = _round_f32r(Wqk)
    wov_r = _round_f32r(Wov)
    xT_r = _round_f32r(xT)  # [D, T]
    x_bf = x.astype(ml_dtypes.bfloat16)

    in_maps = []
    for c in range(NCORES):
        order = [c] + [b for b in range(NKB) if b != c]
        beta_row = np.zeros(NKB, np.float32)
        for slot, b in enumerate(order):
            if b > c:
                beta_row[slot] = NEG
        xqt = _round_f32r(xT[:, c * RQ : (c + 1) * RQ])
        xtp = np.concatenate(
            [xT_r[:, b * 512 : (b + 1) * 512] for b in order], axis=1
        )
        xp = np.concatenate([x_bf[b * 512 : (b + 1) * 512, :] for b in order], axis=0)
        in_maps.append(
            {
                "xqt": np.ascontiguousarray(xqt),
                "xtp": np.ascontiguousarray(xtp),
                "xp": np.ascontiguousarray(xp),
                "wqk": wqk_r,
                "wov": wov_r,
                "beta": np.ascontiguousarray(
                    np.broadcast_to(beta_row, (128, NKB))
                ).astype(np.float32),
            }
        )
    return in_maps


def run(x, Wqk, Wov, **spmd_kwargs):
    """Full pipeline; returns (output [T, D] fp32, BassKernelResults)."""
    import time

    nc = _get_nc()
    in_maps = _prep_in_maps(x, Wqk, Wov)
    try:
        res = run_bass_kernel_spmd(
            nc, in_maps, core_ids=list(range(NCORES)), **spmd_kwargs
        )
    except Exception:
        # a prior crashed execution can leave a core transiently
        # unrecoverable; the runtime resets it — retry once
        time.sleep(10)
        res = run_bass_kernel_spmd(
            nc, in_maps, core_ids=list(range(NCORES)), **spmd_kwargs
        )
    out = np.concatenate([res.results[c]["out"] for c in range(NCORES)], axis=0)
    return np.ascontiguousarray(out, dtype=np.float32), res


def kernel(x, Wqk, Wov):
    out, _ = run(x, Wqk, Wov)
    return out



# revision 3
# speedup vs baseline: 1.2655x; 1.2655x over previous
"""Causal attention kernel for Trainium2, 8 NeuronCores, sequence-parallel.

Reference computation (T=4096, D=1024, fp32):
    q = x @ Wqk; logits = q @ x.T (causal masked); attn = softmax(logits)
    out = (attn @ x) @ Wov

Causal-balanced sharding: global 128-row query tiles i = 0..31 need
keys 0..128(i+1), i.e. w_i = i//4 + 1 key slots of 512. Core c owns
tiles {c, 8+c, 16+c, 24+c} (local m = 0..3, global g = 8m + c), and the
SPMD program gives local tile m a fixed capacity of 2m+2 key slots
(widths 1024/2048/3072/4096). Every core's needs fit exactly:
  c in 0..3: tile m needs 2m+1 slots -> slot 2m is ragged-diagonal,
             slot 2m+1 is fully masked.
  c in 4..7: tile m needs 2m+2 slots -> slot 2m fully visible,
             slot 2m+1 ragged-diagonal.
Keys stay in NATURAL order and are identical on all cores; only the
query-row selection (xqt columns) and two additive mask tiles differ
per core.  maskA applies at slot 2m, maskB at slot 2m+1, for every m:
  c < 4:  maskA = tri(offset 128c),      maskB = all -60000
  c >= 4: maskA = 0,                     maskB = tri(offset 128(c-4))
This cuts score and AV matmul work to 62.5% of the dense version while
keeping one identical instruction stream on all 8 cores.

Precision: fp16 operands (x, Wqk, Wov, q, attn, o1) with fp32 PSUM
accumulation and fp32 softmax stats; masked-out logits get -60000
(fp16-representable; exp underflows to exactly 0). Host-validated
rel_err ~3e-3 (limit 2e-2).
"""

import sys

sys.path.insert(0, "/opt/trn_rl_repo")

import numpy as np

import concourse.tile as tile
from concourse import bacc, mybir
from concourse.bass_utils import run_bass_kernel_spmd

T = 4096
D = 1024
NCORES = 8
RQ = T // NCORES  # 512 query rows per core
NKB = T // 512  # 8 key slots of 512
KC = D // 128  # 8 contraction chunks
NMT = RQ // 128  # 4 query-row tiles per core
CAP = [2 * m + 2 for m in range(NMT)]  # key-slot capacity per local tile
NEG = -60000.0

f32 = mybir.dt.float32
f16 = mybir.dt.float16


def _build_nc():
    nc = bacc.Bacc(
        "TRN2", target_bir_lowering=False, debug=False, num_devices=NCORES
    )

    xqt_d = nc.dram_tensor("xqt", [D, RQ], f16, kind="ExternalInput").ap()
    xtp_d = nc.dram_tensor("xtp", [D, T], f16, kind="ExternalInput").ap()
    xp_d = nc.dram_tensor("xp", [T, D], f16, kind="ExternalInput").ap()
    wqk_d = nc.dram_tensor("wqk", [D, D], f16, kind="ExternalInput").ap()
    wov_d = nc.dram_tensor("wov", [D, D], f16, kind="ExternalInput").ap()
    maska_d = nc.dram_tensor("maska", [128, 512], f16, kind="ExternalInput").ap()
    maskb_d = nc.dram_tensor("maskb", [128, 512], f16, kind="ExternalInput").ap()
    out_d = nc.dram_tensor("out", [RQ, D], f32, kind="ExternalOutput").ap()

    with tile.TileContext(nc) as tc:
        # stack allocator: allocate in order of decreasing lifetime
        consts = tc.alloc_tile_pool(name="consts", bufs=1)
        o1_pool = tc.alloc_tile_pool(name="o1pool", bufs=1)
        pt_pool = tc.alloc_tile_pool(name="ptpool", bufs=1)
        xp_pool = tc.alloc_tile_pool(name="xppool", bufs=1)
        s_pool = tc.alloc_tile_pool(name="spool", bufs=1)
        p_pool = tc.alloc_tile_pool(name="ppool", bufs=4)
        qt_pool = tc.alloc_tile_pool(name="qtpool", bufs=1)

        # constants: masks + stats scratch
        smalls = consts.tile([128, 64], f32, name="smalls")
        negmax = smalls[:, 0:NMT]
        lsum = smalls[:, 4:8]
        recip = smalls[:, 8:12]
        mpart = smalls[:, 12:44]  # [m * NKB + kb]
        lq = smalls[:, 44:60]  # [m * 4 + ch]
        maska = consts.tile([128, 512], f16, name="maska")
        maskb = consts.tile([128, 512], f16, name="maskb")
        nc.sync.dma_start(maska, maska_d)
        nc.sync.dma_start(maskb, maskb_d)

        # long-lived big tiles
        o1t_sb = o1_pool.tile([128, KC * RQ], f16, name="o1t_sb")
        pt_tiles = [
            pt_pool.tile([128, 8 * (m + 1) * 128], f16, name=f"pt_m{m}")
            for m in range(NMT)
        ]
        pt_views = [
            ptm.rearrange("p (kcc q) -> p kcc q", kcc=8 * (m + 1))
            for m, ptm in enumerate(pt_tiles)
        ]
        xp_sb = xp_pool.tile([128, 32 * D], f16, name="xp_sb")
        xp_v = xp_sb.rearrange("p (kc n) -> p kc n", kc=32)
        s_tiles = [
            s_pool.tile([128, 1024 * (m + 1)], f32, name=f"s_m{m}")
            for m in range(NMT)
        ]
        qt_sb = qt_pool.tile([128, KC * RQ], f16, name="qt_sb")

        # ---- Phase A: qT = (xq @ Wqk)^T -> [D, RQ] fp16 ------------------
        # kc-outer with 8 open PSUM chains so compute starts after the
        # first wqk/xqt chunk lands.
        with (
            tc.tile_pool(name="apool", bufs=1) as apool,
            tc.tile_pool(name="psA", bufs=1, space="PSUM") as psA,
        ):
            xqt_sb = apool.tile([128, KC * RQ], f16, name="xqt_sb")
            xqt_v = xqt_sb.rearrange("p (kc n) -> p kc n", kc=KC)
            wqk_sb = apool.tile([128, KC * D], f16, name="wqk_sb")
            wqk_v = wqk_sb.rearrange("p (kc n) -> p kc n", kc=KC)
            xqt_src = xqt_d.rearrange("(kc p) n -> p kc n", p=128)
            ps_a = [psA.tile([128, RQ], f32, name=f"ps_qt{mtd}") for mtd in range(KC)]
            for kc in range(KC):
                nc.sync.dma_start(xqt_v[:, kc, :], xqt_src[:, kc, :])
                nc.sync.dma_start(
                    wqk_v[:, kc, :], wqk_d[kc * 128 : (kc + 1) * 128, :]
                )
                for mtd in range(KC):
                    nc.tensor.matmul(
                        ps_a[mtd][:],
                        wqk_v[:, kc, mtd * 128 : (mtd + 1) * 128],
                        xqt_v[:, kc, :],
                        start=(kc == 0),
                        stop=(kc == KC - 1),
                    )
            for mtd in range(KC):
                nc.vector.tensor_copy(
                    qt_sb[:, mtd * RQ : (mtd + 1) * RQ], ps_a[mtd][:]
                )

        # ---- Phase B: ragged scores + fused softmax prep -----------------
        # slot kb serves local tiles m with CAP[m] > kb; masks at slots
        # 2m (maskA) and 2m+1 (maskB); exp+transpose issued per tile as
        # soon as its last slot completes.
        with (
            tc.tile_pool(name="xtstream", bufs=3) as xtstream,
            tc.tile_pool(name="psB", bufs=2, space="PSUM") as psB,
        ):
            xtp_src = xtp_d.rearrange("p (kb n) -> p kb n", kb=NKB)
            for kb in range(NKB):
                xt = xtstream.tile([128, KC * 512], f16, name="xt", tag="xt")
                xt_v = xt.rearrange("p (kc n) -> p kc n", kc=KC)
                nc.sync.dma_start(
                    xt_v,
                    xtp_src[:, kb, :].rearrange("(kc p) n -> p kc n", p=128),
                )
                # prefetch xp chunks late in the B stream (needed by E)
                if kb >= 4:
                    j = 2 * (kb - 4)
                    for jj in (j, j + 1):
                        nc.sync.dma_start(
                            xp_v[:, 4 * jj : 4 * (jj + 1), :],
                            xp_d[jj * 512 : (jj + 1) * 512, :].rearrange(
                                "(kc p) n -> p kc n", p=128
                            ),
                        )
                for m in range(NMT):
                    if CAP[m] <= kb:
                        continue
                    ps = psB.tile([128, 512], f32, name="ps_s")
                    for kc in range(KC):
                        nc.tensor.matmul(
                            ps[:],
                            qt_sb[:, kc * RQ + m * 128 : kc * RQ + (m + 1) * 128],
                            xt_v[:, kc, :],
                            start=(kc == 0),
                            stop=(kc == KC - 1),
                        )
                    dst = s_tiles[m][:, kb * 512 : (kb + 1) * 512]
                    if kb == 2 * m:
                        nc.vector.tensor_add(dst, ps[:], maska[:])
                    elif kb == 2 * m + 1:
                        nc.vector.tensor_add(dst, ps[:], maskb[:])
                    else:
                        nc.vector.tensor_copy(dst, ps[:])
                    nc.vector.tensor_reduce(
                        mpart[:, m * NKB + kb : m * NKB + kb + 1],
                        dst,
                        axis=mybir.AxisListType.X,
                        op=mybir.AluOpType.max,
                    )
                    if kb == CAP[m] - 1:
                        # tile m complete: finalize stats, exp, transpose
                        nc.vector.tensor_reduce(
                            negmax[:, m : m + 1],
                            mpart[:, m * NKB : m * NKB + CAP[m]],
                            axis=mybir.AxisListType.X,
                            op=mybir.AluOpType.max,
                            negate=True,
                        )
                        for ch in range(m + 1):
                            p_q = p_pool.tile([128, 1024], f16, name="p_q", tag="pq")
                            nc.scalar.activation(
                                p_q[:],
                                s_tiles[m][:, ch * 1024 : (ch + 1) * 1024],
                                mybir.ActivationFunctionType.Exp,
                                bias=negmax[:, m : m + 1],
                                scale=1.0,
                                accum_out=lq[:, m * 4 + ch : m * 4 + ch + 1],
                            )
                            nc.scalar.dma_start_transpose(
                                pt_views[m][:, ch * 8 : (ch + 1) * 8, :], p_q[:]
                            )
                        nc.vector.tensor_reduce(
                            lsum[:, m : m + 1],
                            lq[:, m * 4 : m * 4 + m + 1],
                            axis=mybir.AxisListType.X,
                            op=mybir.AluOpType.add,
                        )
                        nc.vector.reciprocal(
                            recip[:, m : m + 1], lsum[:, m : m + 1]
                        )
        qt_pool.release()

        # ---- Phase E: o1T[:, m] = sum_k x[k,:]^T P[m,k]^T  (ragged) ------
        # m-outer so E(m=0..2) overlaps the exp/transpose tail of m=3.
        wov_pool = tc.alloc_tile_pool(name="wovstream", bufs=2)
        with tc.tile_pool(name="psE", bufs=2, space="PSUM") as psE:
            for m in range(NMT):
                for mtd in range(KC):
                    ps = psE.tile([128, 128], f32, name="ps_av")
                    nk = 8 * (m + 1)
                    for kcc in range(nk):
                        nc.tensor.matmul(
                            ps[:],
                            xp_v[:, kcc, mtd * 128 : (mtd + 1) * 128],
                            pt_views[m][:, kcc, :],
                            start=(kcc == 0),
                            stop=(kcc == nk - 1),
                        )
                    nc.vector.tensor_copy(
                        o1t_sb[:, mtd * RQ + m * 128 : mtd * RQ + (m + 1) * 128],
                        ps[:],
                    )

        # ---- Phase F: out = (o1 @ Wov) * recip ---------------------------
        with (
            tc.tile_pool(name="psF", bufs=2, space="PSUM") as psF,
            tc.tile_pool(name="outp", bufs=3) as outp,
        ):
            for nb in range(2):
                wov_blk = wov_pool.tile([128, KC * 512], f16, name="wov_blk", tag="wv")
                nc.sync.dma_start(
                    wov_blk.rearrange("p (kc n) -> p kc n", kc=KC),
                    wov_d[:, nb * 512 : (nb + 1) * 512].rearrange(
                        "(kc p) n -> p kc n", p=128
                    ),
                )
                for m in range(NMT):
                    ps = psF.tile([128, 512], f32, name="ps_o")
                    for kc in range(KC):
                        nc.tensor.matmul(
                            ps[:],
                            o1t_sb[:, kc * RQ + m * 128 : kc * RQ + (m + 1) * 128],
                            wov_blk[:, kc * 512 : (kc + 1) * 512],
                            start=(kc == 0),
                            stop=(kc == KC - 1),
                        )
                    ob = outp.tile([128, 512], f32, name="ob")
                    nc.vector.tensor_scalar_mul(
                        ob[:], ps[:], recip[:, m : m + 1]
                    )
                    nc.sync.dma_start(
                        out_d[m * 128 : (m + 1) * 128, nb * 512 : (nb + 1) * 512],
                        ob[:],
                    )

        wov_pool.release()
        p_pool.release()
        s_pool.release()
        xp_pool.release()
        pt_pool.release()
        o1_pool.release()
        consts.release()

    nc.compile()
    return nc


_NC_CACHE = {}


def _get_nc():
    if "nc" not in _NC_CACHE:
        _NC_CACHE["nc"] = _build_nc()
    return _NC_CACHE["nc"]


def _prep_in_maps(x, Wqk, Wov):
    x = np.ascontiguousarray(np.asarray(x), dtype=np.float32)
    Wqk = np.ascontiguousarray(np.asarray(Wqk), dtype=np.float32)
    Wov = np.ascontiguousarray(np.asarray(Wov), dtype=np.float32)
    x16 = x.astype(np.float16)
    xtp = np.ascontiguousarray(x16.T)  # [D, T] natural key order
    wqk16 = Wqk.astype(np.float16)
    wov16 = Wov.astype(np.float16)

    p = np.arange(128)[:, None]
    col = np.arange(512)[None, :]

    in_maps = []
    for c in range(NCORES):
        rows = np.concatenate(
            [np.arange(128 * (8 * m + c), 128 * (8 * m + c) + 128) for m in range(NMT)]
        )
        xqt = np.ascontiguousarray(x16[rows, :].T)  # [D, RQ]
        if c < 4:
            maska = np.where(col <= 128 * c + p, 0.0, NEG).astype(np.float16)
            maskb = np.full((128, 512), NEG, np.float16)
        else:
            maska = np.zeros((128, 512), np.float16)
            maskb = np.where(col <= 128 * (c - 4) + p, 0.0, NEG).astype(np.float16)
        in_maps.append(
            {
                "xqt": xqt,
                "xtp": xtp,
                "xp": x16,
                "wqk": wqk16,
                "wov": wov16,
                "maska": np.ascontiguousarray(maska),
                "maskb": np.ascontiguousarray(maskb),
            }
        )
    return in_maps


def run(x, Wqk, Wov, **spmd_kwargs):
    """Full pipeline; returns (output [T, D] fp32, BassKernelResults)."""
    import time

    nc = _get_nc()
    in_maps = _prep_in_maps(x, Wqk, Wov)
    try:
        res = run_bass_kernel_spmd(
            nc, in_maps, core_ids=list(range(NCORES)), **spmd_kwargs
        )
    except Exception:
        # a prior crashed execution can leave a core transiently
        # unrecoverable; the runtime resets it — retry once
        time.sleep(10)
        res = run_bass_kernel_spmd(
            nc, in_maps, core_ids=list(range(NCORES)), **spmd_kwargs
        )
    out = np.empty((T, D), np.float32)
    for c in range(NCORES):
        oc = res.results[c]["out"]
        for m in range(NMT):
            g = 8 * m + c
            out[128 * g : 128 * (g + 1), :] = oc[128 * m : 128 * (m + 1), :]
    return out, res


def kernel(x, Wqk, Wov):
    out, _ = run(x, Wqk, Wov)
    return out


# revision 15
# speedup vs baseline: 1.3825x; 1.0925x over previous
"""Causal attention kernel for Trainium2, 8 NeuronCores, sequence-parallel.

Reference computation (T=4096, D=1024, fp32):
    q = x @ Wqk; logits = q @ x.T (causal masked); attn = softmax(logits)
    out = (attn @ x) @ Wov

Causal-balanced sharding: global 128-row query tiles i = 0..31 need
keys 0..128(i+1), i.e. w_i = i//4 + 1 key slots of 512. Core c owns
tiles {c, 8+c, 16+c, 24+c} (local m = 0..3, global g = 8m + c), and the
SPMD program gives local tile m a fixed capacity of 2m+2 key slots
(widths 1024/2048/3072/4096). Every core's needs fit exactly:
  c in 0..3: tile m needs 2m+1 slots -> slot 2m is ragged-diagonal,
             slot 2m+1 is fully masked.
  c in 4..7: tile m needs 2m+2 slots -> slot 2m fully visible,
             slot 2m+1 ragged-diagonal.
Keys stay in NATURAL order and are identical on all cores; only the
query-row selection (xqt columns) and two additive mask tiles differ
per core.  maskA applies at slot 2m, maskB at slot 2m+1, for every m:
  c < 4:  maskA = tri(offset 128c),      maskB = all -60000
  c >= 4: maskA = 0,                     maskB = tri(offset 128(c-4))
This cuts score and AV matmul work to 62.5% of the dense version while
keeping one identical instruction stream on all 8 cores.

Precision: fp16 operands (x, Wqk, Wov, q, attn, o1) with fp32 PSUM
accumulation and fp32 softmax stats; masked-out logits get -60000
(fp16-representable; exp underflows to exactly 0). Host-validated
rel_err ~3e-3 (limit 2e-2).
"""

import sys

sys.path.insert(0, "/opt/trn_rl_repo")

import numpy as np

import concourse.tile as tile
from concourse import bacc, mybir
from concourse.bass_utils import run_bass_kernel_spmd

T = 4096
D = 1024
NCORES = 8
RQ = T // NCORES  # 512 query rows per core
NKB = T // 512  # 8 key slots of 512
KC = D // 128  # 8 contraction chunks
NMT = RQ // 128  # 4 query-row tiles per core
CAP = [2 * m + 2 for m in range(NMT)]  # key-slot capacity per local tile
NEG = -60000.0

f32 = mybir.dt.float32
f16 = mybir.dt.float16


def _build_nc():
    nc = bacc.Bacc(
        "TRN2", target_bir_lowering=False, debug=False, num_devices=NCORES
    )

    xqt_d = nc.dram_tensor("xqt", [D, RQ], f16, kind="ExternalInput").ap()
    xtp_d = nc.dram_tensor("xtp", [D, T], f16, kind="ExternalInput").ap()
    xp_d = nc.dram_tensor("xp", [T, D], f16, kind="ExternalInput").ap()
    wqk_d = nc.dram_tensor("wqk", [D, D], f16, kind="ExternalInput").ap()
    wov_d = nc.dram_tensor("wov", [D, D], f16, kind="ExternalInput").ap()
    maska_d = nc.dram_tensor("maska", [128, 512], f16, kind="ExternalInput").ap()
    maskb_d = nc.dram_tensor("maskb", [128, 512], f16, kind="ExternalInput").ap()
    out_d = nc.dram_tensor("out", [RQ, D], f32, kind="ExternalOutput").ap()

    with tile.TileContext(nc) as tc:
        # stack allocator: allocate in order of decreasing lifetime
        consts = tc.alloc_tile_pool(name="consts", bufs=1)
        o1_pool = tc.alloc_tile_pool(name="o1pool", bufs=1)
        pt_pool = tc.alloc_tile_pool(name="ptpool", bufs=1)
        xp_pool = tc.alloc_tile_pool(name="xppool", bufs=1)
        s_pool = tc.alloc_tile_pool(name="spool", bufs=1)
        p_pool = tc.alloc_tile_pool(name="ppool", bufs=3)
        qt_pool = tc.alloc_tile_pool(name="qtpool", bufs=1)

        # constants: masks + stats scratch
        smalls = consts.tile([128, 64], f32, name="smalls")
        negmax = smalls[:, 0:NMT]
        lsum = smalls[:, 4:8]
        recip = smalls[:, 8:12]
        mpart = smalls[:, 12:44]  # [m * NKB + kb]
        lq = smalls[:, 44:60]  # [m * 4 + ch]
        maska = consts.tile([128, 512], f16, name="maska")
        maskb = consts.tile([128, 512], f16, name="maskb")

        # long-lived big tiles
        o1t_sb = o1_pool.tile([128, KC * RQ], f16, name="o1t_sb")
        pt_tiles = [
            pt_pool.tile([128, 8 * (m + 1) * 128], f16, name=f"pt_m{m}")
            for m in range(NMT)
        ]
        pt_views = [
            ptm.rearrange("p (kcc q) -> p kcc q", kcc=8 * (m + 1))
            for m, ptm in enumerate(pt_tiles)
        ]
        xp_sb = xp_pool.tile([128, 32 * D], f16, name="xp_sb")
        xp_v = xp_sb.rearrange("p (kc n) -> p kc n", kc=32)
        s_tiles = [
            s_pool.tile([128, 1024 * (m + 1)], f32, name=f"s_m{m}")
            for m in range(NMT)
        ]
        qt_sb = qt_pool.tile([128, KC * RQ], f16, name="qt_sb")

        xtstream = tc.alloc_tile_pool(name="xtstream", bufs=4)

        xtp_src = xtp_d.rearrange("p (kb n) -> p kb n", kb=NKB)
        xt_views = []

        def issue_xt():
            kb = len(xt_views)
            xt = xtstream.tile([128, KC * 512], f16, name="xt", tag="xt")
            xt_v = xt.rearrange("p (kc n) -> p kc n", kc=KC)
            nc.sync.dma_start(
                xt_v, xtp_src[:, kb, :].rearrange("(kc p) n -> p kc n", p=128)
            )
            xt_views.append(xt_v)

        # ---- Phase A: qT = (xq @ Wqk)^T -> [D, RQ] fp16 ------------------
        # mtd-outer: each output chain needs one wqk column block, so the
        # DMA stream stays ahead of the PE. xt/mask loads are interleaved
        # into the spare DMA bandwidth so phase B starts without a stall.
        with (
            tc.tile_pool(name="apool", bufs=1) as apool,
            tc.tile_pool(name="wqkstream", bufs=3) as wqkstream,
            tc.tile_pool(name="psA", bufs=2, space="PSUM") as psA,
        ):
            xqt_sb = apool.tile([128, KC * RQ], f16, name="xqt_sb")
            xqt_v = xqt_sb.rearrange("p (kc n) -> p kc n", kc=KC)
            xqt_src = xqt_d.rearrange("(kc p) n -> p kc n", p=128)
            nc.sync.dma_start(xqt_v[:, 0:4, :], xqt_src[:, 0:4, :])
            for mtd in range(KC):
                wqk_blk = wqkstream.tile([128, KC * 128], f16, name="wqk_blk", tag="wq")
                nc.sync.dma_start(
                    wqk_blk.rearrange("p (kc n) -> p kc n", kc=KC),
                    wqk_d[:, mtd * 128 : (mtd + 1) * 128].rearrange(
                        "(kc p) n -> p kc n", p=128
                    ),
                )
                if mtd == 0:
                    nc.sync.dma_start(xqt_v[:, 4:8, :], xqt_src[:, 4:8, :])
                elif mtd == 1:
                    nc.sync.dma_start(maska, maska_d)
                    nc.sync.dma_start(maskb, maskb_d)
                elif mtd in (3, 5, 7):
                    issue_xt()
                ps = psA.tile([128, RQ], f32, name="ps_qt")
                for kc in range(KC):
                    nc.tensor.matmul(
                        ps[:],
                        wqk_blk[:, kc * 128 : (kc + 1) * 128],
                        xqt_v[:, kc, :],
                        start=(kc == 0),
                        stop=(kc == KC - 1),
                    )
                nc.vector.tensor_copy(
                    qt_sb[:, mtd * RQ : (mtd + 1) * RQ], ps[:]
                )

        # ---- Phase B: ragged scores + fused softmax prep -----------------
        # slot kb serves local tiles m with CAP[m] > kb; masks at slots
        # 2m (maskA) and 2m+1 (maskB); exp+transpose issued per tile as
        # soon as its last slot completes. xp/wov loads ride the late-B
        # DMA shadow, in time for phases E/F.
        with tc.tile_pool(name="psB", bufs=2, space="PSUM") as psB:
            for kb in range(NKB):
                if kb + 3 < NKB:
                    issue_xt()
                else:
                    # xp chunks ride the late-B DMA shadow: 0-3 at kb=5,
                    # 4,5 at kb=6, 6,7 at kb=7
                    chunks = {5: (0, 1, 2, 3), 6: (4, 5), 7: (6, 7)}.get(kb, ())
                    for jj in chunks:
                        nc.sync.dma_start(
                            xp_v[:, 4 * jj : 4 * (jj + 1), :],
                            xp_d[jj * 512 : (jj + 1) * 512, :].rearrange(
                                "(kc p) n -> p kc n", p=128
                            ),
                        )
                xt_v = xt_views[kb]
                for m in range(NMT):
                    if CAP[m] <= kb:
                        continue
                    ps = psB.tile([128, 512], f32, name="ps_s")
                    for kc in range(KC):
                        nc.tensor.matmul(
                            ps[:],
                            qt_sb[:, kc * RQ + m * 128 : kc * RQ + (m + 1) * 128],
                            xt_v[:, kc, :],
                            start=(kc == 0),
                            stop=(kc == KC - 1),
                        )
                    dst = s_tiles[m][:, kb * 512 : (kb + 1) * 512]
                    if kb == 2 * m:
                        nc.vector.tensor_add(dst, ps[:], maska[:])
                    elif kb == 2 * m + 1:
                        nc.vector.tensor_add(dst, ps[:], maskb[:])
                    else:
                        nc.vector.tensor_copy(dst, ps[:])
                    nc.vector.tensor_reduce(
                        mpart[:, m * NKB + kb : m * NKB + kb + 1],
                        dst,
                        axis=mybir.AxisListType.X,
                        op=mybir.AluOpType.max,
                    )
                    if kb == CAP[m] - 1:
                        # tile m complete: finalize stats, exp, transpose
                        nc.vector.tensor_reduce(
                            negmax[:, m : m + 1],
                            mpart[:, m * NKB : m * NKB + CAP[m]],
                            axis=mybir.AxisListType.X,
                            op=mybir.AluOpType.max,
                            negate=True,
                        )
                        for ch in range(m + 1):
                            p_q = p_pool.tile([128, 1024], f16, name="p_q", tag="pq")
                            nc.scalar.activation(
                                p_q[:],
                                s_tiles[m][:, ch * 1024 : (ch + 1) * 1024],
                                mybir.ActivationFunctionType.Exp,
                                bias=negmax[:, m : m + 1],
                                scale=1.0,
                                accum_out=lq[:, m * 4 + ch : m * 4 + ch + 1],
                            )
                            nc.scalar.dma_start_transpose(
                                pt_views[m][:, ch * 8 : (ch + 1) * 8, :], p_q[:]
                            )
                        nc.vector.tensor_reduce(
                            lsum[:, m : m + 1],
                            lq[:, m * 4 : m * 4 + m + 1],
                            axis=mybir.AxisListType.X,
                            op=mybir.AluOpType.add,
                        )
                        nc.vector.reciprocal(
                            recip[:, m : m + 1], lsum[:, m : m + 1]
                        )
        xtstream.release()

        # wov loads reuse the space freed by the xt stream; issued here so
        # they land on the DMA queue after the xp chunks, before F needs them
        wov_pool = tc.alloc_tile_pool(name="wovstream", bufs=1)
        wov_tiles = []
        for nb in range(2):
            wov_blk = wov_pool.tile([128, KC * 512], f16, name=f"wov{nb}")
            wov_tiles.append(wov_blk)
            nc.sync.dma_start(
                wov_blk.rearrange("p (kc n) -> p kc n", kc=KC),
                wov_d[:, nb * 512 : (nb + 1) * 512].rearrange(
                    "(kc p) n -> p kc n", p=128
                ),
            )

        # ---- Phase E: o1T[:, m] = sum_k x[k,:]^T P[m,k]^T  (ragged) ------
        # m-outer so E(m=0..2) overlaps the exp/transpose tail of m=3.
        with tc.tile_pool(name="psE", bufs=2, space="PSUM") as psE:
            for m in range(NMT):
                for mtd in range(KC):
                    ps = psE.tile([128, 128], f32, name="ps_av")
                    nk = 8 * (m + 1)
                    for kcc in range(nk):
                        nc.tensor.matmul(
                            ps[:],
                            xp_v[:, kcc, mtd * 128 : (mtd + 1) * 128],
                            pt_views[m][:, kcc, :],
                            start=(kcc == 0),
                            stop=(kcc == nk - 1),
                        )
                    nc.vector.tensor_copy(
                        o1t_sb[:, mtd * RQ + m * 128 : mtd * RQ + (m + 1) * 128],
                        ps[:],
                    )

        # ---- Phase F: out = (o1 @ Wov) * recip ---------------------------
        with (
            tc.tile_pool(name="psF", bufs=2, space="PSUM") as psF,
            tc.tile_pool(name="outp", bufs=3) as outp,
        ):
            for m in range(NMT):
                for nb in range(2):
                    ps = psF.tile([128, 512], f32, name="ps_o")
                    for kc in range(KC):
                        nc.tensor.matmul(
                            ps[:],
                            o1t_sb[:, kc * RQ + m * 128 : kc * RQ + (m + 1) * 128],
                            wov_tiles[nb][:, kc * 512 : (kc + 1) * 512],
                            start=(kc == 0),
                            stop=(kc == KC - 1),
                        )
                    ob = outp.tile([128, 512], f32, name="ob")
                    nc.vector.tensor_scalar_mul(
                        ob[:], ps[:], recip[:, m : m + 1]
                    )
                    nc.sync.dma_start(
                        out_d[m * 128 : (m + 1) * 128, nb * 512 : (nb + 1) * 512],
                        ob[:],
                    )

        wov_pool.release()
        qt_pool.release()
        p_pool.release()
        # (xtstream already released after phase B)
        s_pool.release()
        xp_pool.release()
        pt_pool.release()
        o1_pool.release()
        consts.release()

    nc.compile()
    return nc


_NC_CACHE = {}


def _get_nc():
    if "nc" not in _NC_CACHE:
        _NC_CACHE["nc"] = _build_nc()
    return _NC_CACHE["nc"]


def _prep_in_maps(x, Wqk, Wov):
    x = np.ascontiguousarray(np.asarray(x), dtype=np.float32)
    Wqk = np.ascontiguousarray(np.asarray(Wqk), dtype=np.float32)
    Wov = np.ascontiguousarray(np.asarray(Wov), dtype=np.float32)
    x16 = x.astype(np.float16)
    xtp = np.ascontiguousarray(x16.T)  # [D, T] natural key order
    wqk16 = Wqk.astype(np.float16)
    wov16 = Wov.astype(np.float16)

    p = np.arange(128)[:, None]
    col = np.arange(512)[None, :]

    in_maps = []
    for c in range(NCORES):
        rows = np.concatenate(
            [np.arange(128 * (8 * m + c), 128 * (8 * m + c) + 128) for m in range(NMT)]
        )
        xqt = np.ascontiguousarray(x16[rows, :].T)  # [D, RQ]
        if c < 4:
            maska = np.where(col <= 128 * c + p, 0.0, NEG).astype(np.float16)
            maskb = np.full((128, 512), NEG, np.float16)
        else:
            maska = np.zeros((128, 512), np.float16)
            maskb = np.where(col <= 128 * (c - 4) + p, 0.0, NEG).astype(np.float16)
        in_maps.append(
            {
                "xqt": xqt,
                "xtp": xtp,
                "xp": x16,
                "wqk": wqk16,
                "wov": wov16,
                "maska": np.ascontiguousarray(maska),
                "maskb": np.ascontiguousarray(maskb),
            }
        )
    return in_maps


def run(x, Wqk, Wov, **spmd_kwargs):
    """Full pipeline; returns (output [T, D] fp32, BassKernelResults)."""
    import time

    nc = _get_nc()
    in_maps = _prep_in_maps(x, Wqk, Wov)
    try:
        res = run_bass_kernel_spmd(
            nc, in_maps, core_ids=list(range(NCORES)), **spmd_kwargs
        )
    except Exception:
        # a prior crashed execution can leave a core transiently
        # unrecoverable; the runtime resets it — retry once
        time.sleep(10)
        res = run_bass_kernel_spmd(
            nc, in_maps, core_ids=list(range(NCORES)), **spmd_kwargs
        )
    out = np.empty((T, D), np.float32)
    for c in range(NCORES):
        oc = res.results[c]["out"]
        for m in range(NMT):
            g = 8 * m + c
            out[128 * g : 128 * (g + 1), :] = oc[128 * m : 128 * (m + 1), :]
    return out, res


def kernel(x, Wqk, Wov):
    out, _ = run(x, Wqk, Wov)
    return out
